# revision 10
# baseline (speedup 1.0000x reference)
"""SE(3) attention block (GNN message passing) on 8 Trainium2 NeuronCores.

Strategy
--------
Edges are sorted by destination node on the host. Nodes are cut into tiles of
(<=128 nodes, <=2048 edges); every tile's edges are padded to exactly 2048
slots (16 blocks of 128 edges). Tiles are distributed contiguously across the
8 cores, so every (node, head) softmax group lives entirely on one core and
inside one tile -> no cross-device collectives at all. The destination-node
query vector for each edge slot is pre-gathered on the host (sorted edges ->
a pure layout transform) and shipped transposed next to k^T.

Per node tile the device kernel:
  1. widens the per-block local-dst columns to a dense [e, b*128] map on ACT
     (one broadcast copy), then builds the one-hot edge->local-node matrix
     with a single whole-tile is_equal compare on DVE (bf16 2x),
  2. prodT = kT * qgT elementwise (one whole-tile DVE op, bf16 2x),
  3. per-head scores via 16 head-mask matmuls (N=8) into one PSUM bank,
  4. ONE fused exp+widen on ACT: reads the PSUM scores with a broadcast AP
     and writes exp(score/sqrt(nf)) replicated 16x (head stride) in bf16,
  5. evex = v * ex_w with one whole-tile DVE 2x multiply into the per-block
     [evex | ex] concat layout; a tiny second ACT exp drops the raw ex into
     the 8 trailing columns of each block,
  6. 16 back-to-back scatter-add matmuls (bf16, N=136) accumulate
     [sum ex*v | sum ex] into a [128, 136] PSUM tile.
The tile is then normalized by 1/sum(exp) and written bf16 into an SBUF
output accumulator; one DMA per 10 tiles ships it out (keeps the Sync
engine's DMA ring free for input prefetch). The normalize ops for tile t
are issued at the top of tile t+1's DVE stream so DVE never stalls waiting
for the PE scatter. GPSIMD is left idle on purpose: concurrent GPSIMD
tensor ops contend with DVE for SBUF ports and inflate both by ~50%.
"""

import math
import numpy as np

# ---------------------------------------------------------------- constants
N_CORES = 8
P = 128                 # partitions / nodes per tile / edges per block
F_BLOCKS = 16           # edge blocks per node tile
EPT = F_BLOCKS * P      # edge slots per tile (2048)
T_PC = 50               # node tiles per core (max 400 total; ~395 needed)
G_OUT = 10              # tiles per output-DMA group
H = 8                   # heads
NF = 128                # features per edge (32*4)
HS = NF // H            # head size (16)
BC = H * HS + H         # per-block scatter rhs cols: [ex*v (128) | ex (8)]
N_NODES = 50000
E_EDGES = 800000
PAD_DST = 300.0         # local-dst sentinel for padding edge slots
INV_SQRT_NF = 1.0 / math.sqrt(NF)

_CACHE = {}
LAST_RESULTS = None     # BassKernelResults of the most recent run (for test.py)


# ---------------------------------------------------------------- device IR
def build_nc(tpc=T_PC, f_blocks=F_BLOCKS):
    """Build the per-core Bass/Tile program (identical on all 8 cores)."""
    from contextlib import ExitStack

    import concourse.bacc as bacc
    import concourse.mybir as mybir
    from concourse.tile import TileContext

    f32 = mybir.dt.float32
    bf16 = mybir.dt.bfloat16
    ept = f_blocks * P
    assert tpc % G_OUT == 0
    n_groups = tpc // G_OUT

    nc = bacc.Bacc("TRN2", target_bir_lowering=False, debug=False)
    kq_d = nc.dram_tensor("kq", [tpc, P, 2 * ept], bf16, kind="ExternalInput")
    vdl_d = nc.dram_tensor("vdl", [tpc, P, ept + f_blocks], bf16,
                           kind="ExternalInput")
    io_d = nc.dram_tensor("iota", [P, ept], bf16, kind="ExternalInput")
    hm_d = nc.dram_tensor("hm", [P, H], bf16, kind="ExternalInput")
    out_d = nc.dram_tensor("out", [n_groups, P, G_OUT * P], bf16,
                           kind="ExternalOutput")

    with TileContext(nc) as tc, ExitStack() as ctx:
        singles = ctx.enter_context(tc.tile_pool(name="singles", bufs=1))
        kqp = ctx.enter_context(tc.tile_pool(name="kqp", bufs=6))
        vdp = ctx.enter_context(tc.tile_pool(name="vdp", bufs=6))
        med = ctx.enter_context(tc.tile_pool(name="med", bufs=2))
        sml = ctx.enter_context(tc.tile_pool(name="sml", bufs=4))
        ps_sc = ctx.enter_context(tc.tile_pool(name="ps_sc", bufs=5, space="PSUM"))
        ps_ag = ctx.enter_context(tc.tile_pool(name="ps_ag", bufs=3, space="PSUM"))

        iota_wide_sb = singles.tile([P, ept], bf16)
        nc.sync.dma_start(out=iota_wide_sb[:], in_=io_d[:, :])
        hm_sb = singles.tile([P, H], bf16)
        nc.sync.dma_start(out=hm_sb[:], in_=hm_d[:, :])

        # Per-tile state, keyed by tile index. The loop below is an explicit
        # software pipeline: at iteration `it` each engine only touches data
        # produced at least one iteration earlier, so no engine ever stalls
        # on another engine's same-iteration output.
        st = {}
        oaccs = {}

        def alloc_and_dma(t):
            g, slot = divmod(t, G_OUT)
            if slot == 0:
                oaccs[g] = med.tile([P, G_OUT * P], bf16, tag="oacc", bufs=2,
                                    name="oacc")
            kq_sb = kqp.tile([P, 2 * ept], bf16, tag="kq")
            nc.sync.dma_start(out=kq_sb[:], in_=kq_d[t])
            vdl_sb = vdp.tile([P, ept + f_blocks], bf16, tag="vdl")
            nc.sync.dma_start(out=vdl_sb[:], in_=vdl_d[t])
            st[t] = {"kq": kq_sb, "vdl": vdl_sb}

        def st_dlw(t):
            # ACT: widen the per-block dst cols to a dense [e, b*128] map
            dlw = med.tile([P, ept], bf16, tag="dlw", bufs=3)
            nc.scalar.copy(
                out=dlw[:].rearrange("p (b n) -> p b n", b=f_blocks),
                in_=st[t]["vdl"][:, ept:ept + f_blocks].to_broadcast(
                    [P, f_blocks, P]),
            )
            st[t]["dlw"] = dlw

        def st_exp(t):
            # ACT: fused exp+widen from PSUM scores, plus raw ex into ev
            s = st[t]
            ex_w = med.tile([P, f_blocks * H * HS], bf16, tag="ex_w", bufs=3)
            nc.scalar.activation(
                out=ex_w[:].rearrange("p (x s) -> p x s", s=HS),
                in_=s["sc"][:].to_broadcast([P, f_blocks * H, HS]),
                func=mybir.ActivationFunctionType.Exp,
                scale=INV_SQRT_NF,
            )
            ev = med.tile([P, f_blocks * BC], bf16, tag="ev", bufs=4)
            nc.scalar.activation(
                out=ev[:].rearrange("p (b c) -> p b c", b=f_blocks)[
                    :, :, H * HS:BC],
                in_=s["sc"][:].rearrange("p (b h) -> p b h", b=f_blocks),
                func=mybir.ActivationFunctionType.Exp,
                scale=INV_SQRT_NF,
            )
            s["ex_w"] = ex_w
            s["ev"] = ev

        def st_front(t):
            # DVE: one-hot compare + kT*qgT product
            s = st[t]
            oh_en = med.tile([P, ept], bf16, tag="oh_en", bufs=4)
            nc.vector.tensor_tensor(
                out=oh_en[:], in0=iota_wide_sb[:], in1=s["dlw"][:],
                op=mybir.AluOpType.is_equal,
            )
            prodT = med.tile([P, ept], bf16, tag="prodT", bufs=3)
            nc.vector.tensor_tensor(
                out=prodT[:], in0=s["kq"][:, 0:ept],
                in1=s["kq"][:, ept:2 * ept],
                op=mybir.AluOpType.mult,
            )
            s["oh"] = oh_en
            s["prodT"] = prodT

        def st_scores(t):
            s = st[t]
            sc_ps = ps_sc.tile([P, f_blocks * H], f32, tag="sc")
            for b in range(f_blocks):
                nc.tensor.matmul(
                    out=sc_ps[:, b * H:(b + 1) * H],
                    lhsT=s["prodT"][:, b * P:(b + 1) * P], rhs=hm_sb[:],
                    start=True, stop=True,
                )
            s["sc"] = sc_ps

        def st_evex(t):
            s = st[t]
            nc.vector.tensor_tensor(
                out=s["ev"][:].rearrange("p (b c) -> p b c", b=f_blocks)[
                    :, :, 0:H * HS],
                in0=s["vdl"][:, 0:ept].rearrange(
                    "p (b c) -> p b c", b=f_blocks),
                in1=s["ex_w"][:].rearrange("p (b c) -> p b c", b=f_blocks),
                op=mybir.AluOpType.mult,
            )

        def st_scatter(t):
            s = st[t]
            agg_ps = ps_ag.tile([P, BC], f32, tag="agg")
            for b in range(f_blocks):
                nc.tensor.matmul(
                    out=agg_ps[:],
                    lhsT=s["oh"][:, b * P:(b + 1) * P],
                    rhs=s["ev"][:, b * BC:(b + 1) * BC],
                    start=(b == 0), stop=(b == f_blocks - 1),
                )
            s["agg"] = agg_ps

        def st_norm(t):
            s = st[t]
            g, slot = divmod(t, G_OUT)
            inv = sml.tile([P, H], f32, tag="inv")
            nc.vector.tensor_scalar(
                out=inv[:], in0=s["agg"][:, H * HS:BC],
                scalar1=1e-30, scalar2=None, op0=mybir.AluOpType.add,
            )
            nc.vector.reciprocal(out=inv[:], in_=inv[:])
            nc.vector.tensor_tensor(
                out=oaccs[g][:, slot * P:(slot + 1) * P].rearrange(
                    "p (h j) -> p h j", h=H),
                in0=s["agg"][:, 0:H * HS].rearrange("p (h j) -> p h j", h=H),
                in1=inv[:].to_broadcast([P, H, HS]),
                op=mybir.AluOpType.mult,
            )
            del st[t]

        def st_ship(g):
            # on the Scalar engine's HW queue: the Sync engine's prefetch
            # stream must never wait behind a compute-dependent DMA
            nc.scalar.dma_start(
                out=out_d[g],
                in_=oaccs[g][:].rearrange("p (j c) -> p j c", j=G_OUT),
            )
            del oaccs[g]

        # Two-iteration scatter skew: the PE's in-order stream would otherwise
        # force scatter(t-1) (which transitively needs scores(t-1) via the
        # exp->evex chain) between scores(t-1) and scores(t), making the
        # cross-engine cycle latency the cadence. With scatter lagging two
        # iterations, every dependency has >= 1 iteration of slack.
        alloc_and_dma(0)
        for it in range(tpc + 4):
            if it + 1 < tpc:
                alloc_and_dma(it + 1)
            if it == 0:
                st_dlw(0)
            if 0 <= it - 1 < tpc:
                st_exp(it - 1)
            if it < tpc:
                st_front(it)
                st_scores(it)
            if 0 <= it - 1 < tpc:
                st_evex(it - 1)
            if 0 <= it - 2 < tpc:
                st_scatter(it - 2)
            if it + 1 < tpc:
                st_dlw(it + 1)
            if 0 <= it - 3 < tpc:
                st_norm(it - 3)
            if it >= 13 and (it - 13) % G_OUT == 0 and (it - 13) // G_OUT < n_groups:
                st_ship((it - 13) // G_OUT)
    nc.compile()
    return nc


# ------------------------------------------------------------ host plumbing
def _build_tiles(cum, n_nodes, ept):
    """Greedy cut of nodes into (<=128 nodes, <=ept edges) tiles."""
    tiles = []
    n0 = 0
    while n0 < n_nodes:
        n1 = int(np.searchsorted(cum, cum[n0] + ept, side="right")) - 1
        n1 = min(n1, n0 + P, n_nodes)
        if n1 <= n0:
            raise ValueError(f"node {n0} has degree > {ept}; unsupported")
        tiles.append((n0, n1))
        n0 = n1
    return tiles


def _prep_inputs(value, key, query_0, query_1, edge_index,
                 tpc=T_PC, f_blocks=F_BLOCKS, n_cores=N_CORES):
    """Sort/tile/pad on the host; returns per-core input maps + assembly info."""
    import ml_dtypes
    bf16 = ml_dtypes.bfloat16

    ept = f_blocks * P
    value = np.ascontiguousarray(np.asarray(value, dtype=np.float32))
    key = np.ascontiguousarray(np.asarray(key, dtype=np.float32))
    q0 = np.asarray(query_0, dtype=np.float32)
    q1 = np.asarray(query_1, dtype=np.float32)
    ei = np.asarray(edge_index)
    n_nodes = q0.shape[0]
    n_edges = key.shape[0]

    dst = ei[1].astype(np.int64).ravel()
    order = np.argsort(dst, kind="stable")
    dsts = dst[order]
    counts = np.bincount(dsts, minlength=n_nodes)
    cum = np.zeros(n_nodes + 1, np.int64)
    cum[1:] = np.cumsum(counts)

    tiles = _build_tiles(cum, n_nodes, ept)
    t_total = len(tiles)
    if t_total > n_cores * tpc:
        raise ValueError(f"{t_total} tiles > capacity {n_cores * tpc}")
    q_per_core = (t_total + n_cores - 1) // n_cores  # real tiles per core
    t8 = n_cores * tpc

    # slot -> original edge id (or padding), per global tile slot
    slot_edge = np.full((t8, ept), 0, np.int64)
    slot_valid = np.zeros((t8, ept), bool)
    slot_dst = np.full((t8, ept), 0, np.int64)   # global dst per slot
    dl = np.full((t8, ept), PAD_DST, np.float32)
    tile_info = []  # (global_tile_idx, n0, n_cnt)
    for i, (n0, n1) in enumerate(tiles):
        c, j = divmod(i, q_per_core)
        idx = c * tpc + j
        e0, e1 = int(cum[n0]), int(cum[n1])
        cnt = e1 - e0
        slot_edge[idx, :cnt] = order[e0:e1]
        slot_valid[idx, :cnt] = True
        slot_dst[idx, :cnt] = dsts[e0:e1]
        dl[idx, :cnt] = (dsts[e0:e1] - n0).astype(np.float32)
        tile_info.append((idx, n0, n1 - n0))

    flat_edge = slot_edge.reshape(-1)
    flat_valid = slot_valid.reshape(-1)

    kf = key.reshape(n_edges, NF)
    k_slots = kf[flat_edge]
    k_slots[~flat_valid] = 0.0
    q_cat = np.concatenate([q0, q1], axis=-1).reshape(
        n_nodes, NF).astype(np.float32)
    qg_slots = q_cat[slot_dst.reshape(-1)]
    qg_slots[~flat_valid] = 0.0
    # merged [kT | qgT]: [t, f, b*128+e] twice, bf16
    kq = np.empty((t8, NF, 2 * ept), bf16)
    kq[:, :, :ept] = k_slots.reshape(
        t8, f_blocks, P, NF).transpose(0, 3, 1, 2).reshape(t8, NF, ept)
    kq[:, :, ept:] = qg_slots.reshape(
        t8, f_blocks, P, NF).transpose(0, 3, 1, 2).reshape(t8, NF, ept)
    del k_slots, qg_slots

    vf = value.reshape(n_edges, NF)
    v_slots = vf[flat_edge]
    v_slots[~flat_valid] = 0.0
    # natural per-block v: [t, e, b*128+f], plus the per-block local-dst cols
    vdl = np.empty((t8, P, ept + f_blocks), np.float32)
    vdl[:, :, :ept] = v_slots.reshape(t8, f_blocks, P, NF).transpose(
        0, 2, 1, 3).reshape(t8, P, ept)
    del v_slots
    vdl[:, :, ept:] = dl.reshape(t8, f_blocks, P).transpose(0, 2, 1)
    vdl = vdl.astype(bf16)

    iota = np.broadcast_to(np.arange(P, dtype=np.float32)[None, None, :],
                           (P, f_blocks, P)).reshape(P, ept).astype(bf16)
    hm = np.zeros((NF, H), np.float32)
    for h in range(H):
        hm[h * HS:(h + 1) * HS, h] = 1.0
    hm = hm.astype(bf16)

    in_maps = []
    for c in range(n_cores):
        s = slice(c * tpc, (c + 1) * tpc)
        in_maps.append({
            "kq": kq[s], "vdl": vdl[s], "iota": iota, "hm": hm,
        })
    return in_maps, tile_info, n_nodes


def _assemble(results, tile_info, n_nodes, tpc=T_PC):
    out = np.zeros((n_nodes, NF), np.float32)
    per_core = []
    for c in range(len(results)):
        o = np.asarray(results[c]["out"], dtype=np.float32)
        n_groups = o.shape[0]
        # [g, p, j*128+c] -> [g*G+j, p, c]
        o = o.reshape(n_groups, P, G_OUT, P).transpose(0, 2, 1, 3).reshape(
            n_groups * G_OUT, P, P)
        per_core.append(o)
    for idx, n0, cnt in tile_info:
        c, j = divmod(idx, tpc)
        out[n0:n0 + cnt] = per_core[c][j, :cnt]
    return out.reshape(n_nodes, NF // 4, 4)


def _get_nc(tpc=T_PC, f_blocks=F_BLOCKS):
    key = (tpc, f_blocks)
    if key not in _CACHE:
        _CACHE[key] = build_nc(tpc, f_blocks)
    return _CACHE[key]


def _needed_tpc(edge_index, n_nodes, ept, n_cores=N_CORES):
    dst = np.asarray(edge_index)[1].astype(np.int64).ravel()
    counts = np.bincount(dst, minlength=n_nodes)
    cum = np.zeros(n_nodes + 1, np.int64)
    cum[1:] = np.cumsum(counts)
    t_total = len(_build_tiles(cum, n_nodes, ept))
    return (t_total + n_cores - 1) // n_cores


def _run(inputs, trace=False, tpc=T_PC, f_blocks=F_BLOCKS, **spmd_kwargs):
    global LAST_RESULTS
    from concourse.bass_utils import run_bass_kernel_spmd

    tpc = max(tpc, _needed_tpc(inputs["edge_index"],
                               np.asarray(inputs["query_0"]).shape[0],
                               f_blocks * P))
    tpc = ((tpc + G_OUT - 1) // G_OUT) * G_OUT
    nc = _get_nc(tpc, f_blocks)
    in_maps, tile_info, n_nodes = _prep_inputs(
        inputs["value"], inputs["key"], inputs["query_0"], inputs["query_1"],
        inputs["edge_index"], tpc=tpc, f_blocks=f_blocks)
    res = run_bass_kernel_spmd(
        nc, in_maps, list(range(N_CORES)), trace=trace, **spmd_kwargs)
    LAST_RESULTS = res
    return _assemble(res.results, tile_info, n_nodes, tpc=tpc)


def kernel(value, key, query_0, query_1, edge_index):
    return _run({
        "value": value, "key": key, "query_0": query_0,
        "query_1": query_1, "edge_index": edge_index,
    })


# revision 14
# speedup vs baseline: 1.1634x; 1.1634x over previous
"""SE(3) attention block (GNN message passing) on 8 Trainium2 NeuronCores.

Strategy
--------
Edges are sorted by destination node on the host. Nodes are cut into tiles of
(<=128 nodes, <=2048 edges); every tile's edges are padded to exactly 2048
slots (16 blocks of 128 edges). Tiles are distributed contiguously across the
8 cores, so every (node, head) softmax group lives entirely on one core and
inside one tile -> no cross-device collectives at all. The destination-node
query vector for each edge slot is pre-gathered on the host (sorted edges ->
a pure layout transform) and shipped transposed next to k^T.

Per node tile the device kernel:
  1. widens the per-block local-dst columns to a dense [e, b*128] map on ACT
     (one broadcast copy), then builds the one-hot edge->local-node matrix
     with a single whole-tile is_equal compare on DVE (bf16 2x),
  2. prodT = kT * qgT elementwise (one whole-tile DVE op, bf16 2x),
  3. per-head scores via 16 head-mask matmuls (N=8) into one PSUM bank,
  4. ONE fused exp+widen on ACT: reads the PSUM scores with a broadcast AP
     and writes exp(score/sqrt(nf)) replicated 16x (head stride) in bf16,
  5. evex = v * ex_w with one whole-tile DVE 2x multiply into the per-block
     [evex | ex] concat layout; a tiny second ACT exp drops the raw ex into
     the 8 trailing columns of each block,
  6. 16 back-to-back scatter-add matmuls (bf16, N=136) accumulate
     [sum ex*v | sum ex] into a [128, 136] PSUM tile.
The tile is then normalized by 1/sum(exp) and written bf16 into an SBUF
output accumulator; one DMA per 10 tiles ships it out (keeps the Sync
engine's DMA ring free for input prefetch). The normalize ops for tile t
are issued at the top of tile t+1's DVE stream so DVE never stalls waiting
for the PE scatter. GPSIMD is left idle on purpose: concurrent GPSIMD
tensor ops contend with DVE for SBUF ports and inflate both by ~50%.
"""

import math
import numpy as np

# ---------------------------------------------------------------- constants
N_CORES = 8
P = 128                 # partitions / nodes per tile / edges per block
F_BLOCKS = 16           # edge blocks per node tile
EPT = F_BLOCKS * P      # edge slots per tile (2048)
T_PC = 50               # node tiles per core (max 400 total; ~395 needed)
G_OUT = 10              # tiles per output-DMA group
H = 8                   # heads
NF = 128                # features per edge (32*4)
HS = NF // H            # head size (16)
BC = H * HS + H         # per-block scatter rhs cols: [ex*v (128) | ex (8)]
N_NODES = 50000
E_EDGES = 800000
PAD_DST = 300.0         # local-dst sentinel for padding edge slots
INV_SQRT_NF = 1.0 / math.sqrt(NF)

_CACHE = {}
LAST_RESULTS = None     # BassKernelResults of the most recent run (for test.py)


# ---------------------------------------------------------------- device IR
def build_nc(tpc=T_PC, f_blocks=F_BLOCKS):
    """Build the per-core Bass/Tile program (identical on all 8 cores)."""
    from contextlib import ExitStack

    import concourse.bacc as bacc
    import concourse.mybir as mybir
    from concourse.tile import TileContext

    f32 = mybir.dt.float32
    bf16 = mybir.dt.bfloat16
    ept = f_blocks * P
    assert tpc % G_OUT == 0
    n_groups = tpc // G_OUT

    nc = bacc.Bacc("TRN2", target_bir_lowering=False, debug=False)
    kq_d = nc.dram_tensor("kq", [tpc, P, 2 * ept], bf16, kind="ExternalInput")
    vdl_d = nc.dram_tensor("vdl", [tpc, P, ept + f_blocks], bf16,
                           kind="ExternalInput")
    io_d = nc.dram_tensor("iota", [P, ept], bf16, kind="ExternalInput")
    hm_d = nc.dram_tensor("hm", [P, H], bf16, kind="ExternalInput")
    out_d = nc.dram_tensor("out", [n_groups, P, G_OUT * P], bf16,
                           kind="ExternalOutput")

    with TileContext(nc) as tc, ExitStack() as ctx:
        singles = ctx.enter_context(tc.tile_pool(name="singles", bufs=1))
        kqp = ctx.enter_context(tc.tile_pool(name="kqp", bufs=6))
        vdp = ctx.enter_context(tc.tile_pool(name="vdp", bufs=6))
        med = ctx.enter_context(tc.tile_pool(name="med", bufs=2))
        sml = ctx.enter_context(tc.tile_pool(name="sml", bufs=4))
        ps_sc = ctx.enter_context(tc.tile_pool(name="ps_sc", bufs=5, space="PSUM"))
        ps_ag = ctx.enter_context(tc.tile_pool(name="ps_ag", bufs=3, space="PSUM"))

        iota_wide_sb = singles.tile([P, ept], bf16)
        nc.sync.dma_start(out=iota_wide_sb[:], in_=io_d[:, :])
        hm_sb = singles.tile([P, H], bf16)
        nc.sync.dma_start(out=hm_sb[:], in_=hm_d[:, :])

        # Per-tile state, keyed by tile index. The loop below is an explicit
        # software pipeline: at iteration `it` each engine only touches data
        # produced at least one iteration earlier, so no engine ever stalls
        # on another engine's same-iteration output.
        st = {}
        oaccs = {}

        def alloc_and_dma(t):
            g, slot = divmod(t, G_OUT)
            if slot == 0:
                oaccs[g] = med.tile([P, G_OUT * P], bf16, tag="oacc", bufs=2,
                                    name="oacc")
            kq_sb = kqp.tile([P, 2 * ept], bf16, tag="kq")
            nc.sync.dma_start(out=kq_sb[:], in_=kq_d[t])
            vdl_sb = vdp.tile([P, ept + f_blocks], bf16, tag="vdl")
            nc.sync.dma_start(out=vdl_sb[:], in_=vdl_d[t])
            st[t] = {"kq": kq_sb, "vdl": vdl_sb}

        def st_dlw(t):
            # ACT: widen the per-block dst cols to a dense [e, b*128] map
            dlw = med.tile([P, ept], bf16, tag="dlw", bufs=3)
            nc.scalar.copy(
                out=dlw[:].rearrange("p (b n) -> p b n", b=f_blocks),
                in_=st[t]["vdl"][:, ept:ept + f_blocks].to_broadcast(
                    [P, f_blocks, P]),
            )
            st[t]["dlw"] = dlw

        def st_exp(t, half):
            # ACT: fused exp+widen from PSUM scores for 8 of the 16 blocks
            s = st[t]
            if half == 0:
                s["ex_w"] = med.tile([P, f_blocks * H * HS], bf16,
                                     tag="ex_w", bufs=3, name="ex_w")
                s["ev"] = med.tile([P, f_blocks * BC], bf16, tag="ev",
                                   bufs=3, name="ev")
            hb = f_blocks // 2
            xc = slice(half * hb * H, (half + 1) * hb * H)
            nc.scalar.activation(
                out=s["ex_w"][:, half * hb * H * HS:(half + 1) * hb * H * HS
                              ].rearrange("p (x s) -> p x s", s=HS),
                in_=s["sc"][:, xc].to_broadcast([P, hb * H, HS]),
                func=mybir.ActivationFunctionType.Exp,
                scale=INV_SQRT_NF,
            )

        def st_excopy(t, half):
            # ACT: drop the raw ex into the 8 trailing cols of each block's
            # scatter rhs (exp straight from the PSUM scores)
            s = st[t]
            hb = f_blocks // 2
            bsl = slice(half * hb, (half + 1) * hb)
            nc.scalar.activation(
                out=s["ev"][:].rearrange("p (b c) -> p b c", b=f_blocks)[
                    :, bsl, H * HS:BC],
                in_=s["sc"][:].rearrange("p (b h) -> p b h",
                                         b=f_blocks)[:, bsl],
                func=mybir.ActivationFunctionType.Exp,
                scale=INV_SQRT_NF,
            )

        def st_front(t):
            # DVE: one-hot compare + kT*qgT product
            s = st[t]
            oh_en = med.tile([P, ept], bf16, tag="oh_en", bufs=4)
            nc.vector.tensor_tensor(
                out=oh_en[:], in0=iota_wide_sb[:], in1=s["dlw"][:],
                op=mybir.AluOpType.is_equal,
            )
            prodT = med.tile([P, ept], bf16, tag="prodT", bufs=3)
            nc.vector.tensor_tensor(
                out=prodT[:], in0=s["kq"][:, 0:ept],
                in1=s["kq"][:, ept:2 * ept],
                op=mybir.AluOpType.mult,
            )
            s["oh"] = oh_en
            s["prodT"] = prodT

        def st_scores(t):
            s = st[t]
            sc_ps = ps_sc.tile([P, f_blocks * H], f32, tag="sc")
            for b in range(f_blocks):
                nc.tensor.matmul(
                    out=sc_ps[:, b * H:(b + 1) * H],
                    lhsT=s["prodT"][:, b * P:(b + 1) * P], rhs=hm_sb[:],
                    start=True, stop=True,
                )
            s["sc"] = sc_ps

        def st_evex(t, half):
            s = st[t]
            hb = f_blocks // 2
            bsl = slice(half * hb, (half + 1) * hb)
            nc.vector.tensor_tensor(
                out=s["ev"][:].rearrange("p (b c) -> p b c", b=f_blocks)[
                    :, bsl, 0:H * HS],
                in0=s["vdl"][:, 0:ept].rearrange(
                    "p (b c) -> p b c", b=f_blocks)[:, bsl],
                in1=s["ex_w"][:].rearrange("p (b c) -> p b c",
                                           b=f_blocks)[:, bsl],
                op=mybir.AluOpType.mult,
            )

        def st_scatter(t):
            s = st[t]
            agg_ps = ps_ag.tile([P, BC], f32, tag="agg")
            for b in range(f_blocks):
                nc.tensor.matmul(
                    out=agg_ps[:],
                    lhsT=s["oh"][:, b * P:(b + 1) * P],
                    rhs=s["ev"][:, b * BC:(b + 1) * BC],
                    start=(b == 0), stop=(b == f_blocks - 1),
                )
            s["agg"] = agg_ps

        def st_norm(t):
            s = st[t]
            g, slot = divmod(t, G_OUT)
            inv = sml.tile([P, H], f32, tag="inv")
            nc.vector.tensor_scalar(
                out=inv[:], in0=s["agg"][:, H * HS:BC],
                scalar1=1e-30, scalar2=None, op0=mybir.AluOpType.add,
            )
            nc.vector.reciprocal(out=inv[:], in_=inv[:])
            nc.vector.tensor_tensor(
                out=oaccs[g][:, slot * P:(slot + 1) * P].rearrange(
                    "p (h j) -> p h j", h=H),
                in0=s["agg"][:, 0:H * HS].rearrange("p (h j) -> p h j", h=H),
                in1=inv[:].to_broadcast([P, H, HS]),
                op=mybir.AluOpType.mult,
            )
            del st[t]

        def st_ship(g):
            # on the Scalar engine's HW queue: the Sync engine's prefetch
            # stream must never wait behind a compute-dependent DMA
            nc.scalar.dma_start(
                out=out_d[g],
                in_=oaccs[g][:].rearrange("p (j c) -> p j c", j=G_OUT),
            )
            del oaccs[g]

        # Natural per-tile emission (the Tile scheduler's priority order).
        # The exp->evex->scatter chain is split into block-halves so the
        # engines pipeline within a tile instead of serializing on the
        # whole-tile ops.
        for t in range(tpc):
            alloc_and_dma(t)
            if t >= 1:
                st_norm(t - 1)
            g, slot = divmod(t, G_OUT)
            if slot == 2 and g > 0:
                st_ship(g - 1)
            st_dlw(t)
            st_front(t)
            st_scores(t)
            st_exp(t, 0)
            st_evex(t, 0)
            st_excopy(t, 0)
            st_exp(t, 1)
            st_evex(t, 1)
            st_excopy(t, 1)
            st_scatter(t)
        st_norm(tpc - 1)
        st_ship(n_groups - 1)
    nc.compile()
    return nc


# ------------------------------------------------------------ host plumbing
def _build_tiles(cum, n_nodes, ept):
    """Greedy cut of nodes into (<=128 nodes, <=ept edges) tiles."""
    tiles = []
    n0 = 0
    while n0 < n_nodes:
        n1 = int(np.searchsorted(cum, cum[n0] + ept, side="right")) - 1
        n1 = min(n1, n0 + P, n_nodes)
        if n1 <= n0:
            raise ValueError(f"node {n0} has degree > {ept}; unsupported")
        tiles.append((n0, n1))
        n0 = n1
    return tiles


def _prep_inputs(value, key, query_0, query_1, edge_index,
                 tpc=T_PC, f_blocks=F_BLOCKS, n_cores=N_CORES):
    """Sort/tile/pad on the host; returns per-core input maps + assembly info."""
    import ml_dtypes
    bf16 = ml_dtypes.bfloat16

    ept = f_blocks * P
    value = np.ascontiguousarray(np.asarray(value, dtype=np.float32))
    key = np.ascontiguousarray(np.asarray(key, dtype=np.float32))
    q0 = np.asarray(query_0, dtype=np.float32)
    q1 = np.asarray(query_1, dtype=np.float32)
    ei = np.asarray(edge_index)
    n_nodes = q0.shape[0]
    n_edges = key.shape[0]

    dst = ei[1].astype(np.int64).ravel()
    order = np.argsort(dst, kind="stable")
    dsts = dst[order]
    counts = np.bincount(dsts, minlength=n_nodes)
    cum = np.zeros(n_nodes + 1, np.int64)
    cum[1:] = np.cumsum(counts)

    tiles = _build_tiles(cum, n_nodes, ept)
    t_total = len(tiles)
    if t_total > n_cores * tpc:
        raise ValueError(f"{t_total} tiles > capacity {n_cores * tpc}")
    q_per_core = (t_total + n_cores - 1) // n_cores  # real tiles per core
    t8 = n_cores * tpc

    # slot -> original edge id (or padding), per global tile slot
    slot_edge = np.full((t8, ept), 0, np.int64)
    slot_valid = np.zeros((t8, ept), bool)
    slot_dst = np.full((t8, ept), 0, np.int64)   # global dst per slot
    dl = np.full((t8, ept), PAD_DST, np.float32)
    tile_info = []  # (global_tile_idx, n0, n_cnt)
    for i, (n0, n1) in enumerate(tiles):
        c, j = divmod(i, q_per_core)
        idx = c * tpc + j
        e0, e1 = int(cum[n0]), int(cum[n1])
        cnt = e1 - e0
        slot_edge[idx, :cnt] = order[e0:e1]
        slot_valid[idx, :cnt] = True
        slot_dst[idx, :cnt] = dsts[e0:e1]
        dl[idx, :cnt] = (dsts[e0:e1] - n0).astype(np.float32)
        tile_info.append((idx, n0, n1 - n0))

    flat_edge = slot_edge.reshape(-1)
    flat_valid = slot_valid.reshape(-1)

    kf = key.reshape(n_edges, NF)
    k_slots = kf[flat_edge]
    k_slots[~flat_valid] = 0.0
    q_cat = np.concatenate([q0, q1], axis=-1).reshape(
        n_nodes, NF).astype(np.float32)
    qg_slots = q_cat[slot_dst.reshape(-1)]
    qg_slots[~flat_valid] = 0.0
    # merged [kT | qgT]: [t, f, b*128+e] twice, bf16
    kq = np.empty((t8, NF, 2 * ept), bf16)
    kq[:, :, :ept] = k_slots.reshape(
        t8, f_blocks, P, NF).transpose(0, 3, 1, 2).reshape(t8, NF, ept)
    kq[:, :, ept:] = qg_slots.reshape(
        t8, f_blocks, P, NF).transpose(0, 3, 1, 2).reshape(t8, NF, ept)
    del k_slots, qg_slots

    vf = value.reshape(n_edges, NF)
    v_slots = vf[flat_edge]
    v_slots[~flat_valid] = 0.0
    # natural per-block v: [t, e, b*128+f], plus the per-block local-dst cols
    vdl = np.empty((t8, P, ept + f_blocks), np.float32)
    vdl[:, :, :ept] = v_slots.reshape(t8, f_blocks, P, NF).transpose(
        0, 2, 1, 3).reshape(t8, P, ept)
    del v_slots
    vdl[:, :, ept:] = dl.reshape(t8, f_blocks, P).transpose(0, 2, 1)
    vdl = vdl.astype(bf16)

    iota = np.broadcast_to(np.arange(P, dtype=np.float32)[None, None, :],
                           (P, f_blocks, P)).reshape(P, ept).astype(bf16)
    hm = np.zeros((NF, H), np.float32)
    for h in range(H):
        hm[h * HS:(h + 1) * HS, h] = 1.0
    hm = hm.astype(bf16)

    in_maps = []
    for c in range(n_cores):
        s = slice(c * tpc, (c + 1) * tpc)
        in_maps.append({
            "kq": kq[s], "vdl": vdl[s], "iota": iota, "hm": hm,
        })
    return in_maps, tile_info, n_nodes


def _assemble(results, tile_info, n_nodes, tpc=T_PC):
    out = np.zeros((n_nodes, NF), np.float32)
    per_core = []
    for c in range(len(results)):
        o = np.asarray(results[c]["out"], dtype=np.float32)
        n_groups = o.shape[0]
        # [g, p, j*128+c] -> [g*G+j, p, c]
        o = o.reshape(n_groups, P, G_OUT, P).transpose(0, 2, 1, 3).reshape(
            n_groups * G_OUT, P, P)
        per_core.append(o)
    for idx, n0, cnt in tile_info:
        c, j = divmod(idx, tpc)
        out[n0:n0 + cnt] = per_core[c][j, :cnt]
    return out.reshape(n_nodes, NF // 4, 4)


def _get_nc(tpc=T_PC, f_blocks=F_BLOCKS):
    key = (tpc, f_blocks)
    if key not in _CACHE:
        _CACHE[key] = build_nc(tpc, f_blocks)
    return _CACHE[key]


def _needed_tpc(edge_index, n_nodes, ept, n_cores=N_CORES):
    dst = np.asarray(edge_index)[1].astype(np.int64).ravel()
    counts = np.bincount(dst, minlength=n_nodes)
    cum = np.zeros(n_nodes + 1, np.int64)
    cum[1:] = np.cumsum(counts)
    t_total = len(_build_tiles(cum, n_nodes, ept))
    return (t_total + n_cores - 1) // n_cores


def _run(inputs, trace=False, tpc=T_PC, f_blocks=F_BLOCKS, **spmd_kwargs):
    global LAST_RESULTS
    from concourse.bass_utils import run_bass_kernel_spmd

    tpc = max(tpc, _needed_tpc(inputs["edge_index"],
                               np.asarray(inputs["query_0"]).shape[0],
                               f_blocks * P))
    tpc = ((tpc + G_OUT - 1) // G_OUT) * G_OUT
    nc = _get_nc(tpc, f_blocks)
    in_maps, tile_info, n_nodes = _prep_inputs(
        inputs["value"], inputs["key"], inputs["query_0"], inputs["query_1"],
        inputs["edge_index"], tpc=tpc, f_blocks=f_blocks)
    res = run_bass_kernel_spmd(
        nc, in_maps, list(range(N_CORES)), trace=trace, **spmd_kwargs)
    LAST_RESULTS = res
    return _assemble(res.results, tile_info, n_nodes, tpc=tpc)


def kernel(value, key, query_0, query_1, edge_index):
    return _run({
        "value": value, "key": key, "query_0": query_0,
        "query_1": query_1, "edge_index": edge_index,
    })
